# revision 17
# baseline (speedup 1.0000x reference)
"""ALiBi attention (B=2, S=2048, H=1024, 16 heads, d=64, f32) on 8 TRN2 cores.

Sharding: heads (2 per core) for qkv-proj + attention; AllToAll per batch
switches to sequence sharding (256 tokens per core) for the output projection.

v2: all matmuls bf16 (FastWeightLoad + 1 cyc/row), and the ALiBi bias is
applied MULTIPLICATIVELY after exp:  probs = exp(scores + mask) * E  where
E_h = exp(-slope_h * |i-j|) is a precomputed per-head Toeplitz table
E[p, m] = exp(-slope|p - m + 1920|), viewed at offset 1920 - 128*kc + q0 for
each (key-chunk, q-span) tile. exp runs on ACT straight out of PSUM (721ns)
and the bf16 SBUF multiply runs on DVE at 4x mode (335ns) — vs the v1
PSUM-reading scalar_tensor_tensor at 812-3900ns.

Other structure per core:
  - qT/kT per (batch, head) [64, 2048] bf16; vT per head [65, 4096] bf16 with
    a ones row (becomes the ones column of v_aug -> PV matmul also emits the
    softmax normalizer in row 64).
  - attention mask folded in as a per-partition additive exp bias (0/-3e4).
  - normalize rows 0..63 by reciprocal_approx_fast(row 64), broadcast across
    partitions with a ones[1,64] fp32 matmul (is_stationary_onezero).
  - AllToAll (bf16 payload, one per batch) redistributes head-dims -> tokens;
    out-proj yT[o, tok] accumulated over the 8 src dim-blocks, + out_b.
"""
import math
import os
import sys
import types
import numpy as np
import ml_dtypes

B = 2
S = 2048
H = 1024
HEADS = 16
D = 64
N_CORES = 8
HPC = HEADS // N_CORES          # heads per core = 2
TOK = B * S                     # 4096
SPAN = 512
NSPAN = TOK // SPAN             # 8 proj spans (batch-major)
QSPANS = S // SPAN              # 4 q spans per batch
KC = S // 128                   # 16 key chunks per batch
RM0 = 1920                      # Toeplitz table center offset
RCOLS = 3968
SCALE = D ** -0.5
MASK_NEG = -30000.0
BFNP = ml_dtypes.bfloat16


def _install_ntff_hook():
    """The agent image's antenv lacks axon_hooks; register the NTFF profiling
    hook ourselves so run_bass_kernel_spmd(trace=True) works under axon."""
    if "antenv.axon_hooks" in sys.modules:
        return
    try:
        sys.path.insert(0, "/root/.axon_site")
        from trn_agent_boot.trn_boot import _ntff_profile_via_ctypes
        hook = _ntff_profile_via_ctypes("/opt/axon/libaxon_pjrt.so")
        mod = types.ModuleType("antenv.axon_hooks")
        mod.get_axon_ntff_profile_hook = lambda: hook
        mod.set_axon_ntff_profile_hook = lambda h: None
        import antenv
        antenv.axon_hooks = mod
        sys.modules["antenv.axon_hooks"] = mod
    except Exception:
        pass


def get_alibi_slopes(num_heads):
    def pow2_slopes(n):
        start = 2 ** (-(2 ** (-(math.log2(n) - 3))))
        return [start * start ** i for i in range(n)]
    if math.log2(num_heads).is_integer():
        return pow2_slopes(num_heads)
    c = 2 ** math.floor(math.log2(num_heads))
    return pow2_slopes(c) + pow2_slopes(2 * c)[: num_heads - c]


SLOPES = get_alibi_slopes(HEADS)


def build_nc():
    import concourse.mybir as mybir
    import concourse.bacc as bacc
    import concourse.tile as tile

    F32 = mybir.dt.float32
    BF16 = mybir.dt.bfloat16
    Exp = mybir.ActivationFunctionType.Exp
    Ident = mybir.ActivationFunctionType.Identity

    nc = bacc.Bacc(None, target_bir_lowering=False)

    xt = nc.declare_dram_parameter("xt", [H, TOK], BF16, isOutput=False)
    wqkt = nc.declare_dram_parameter("wqkt", [H, 256], BF16, isOutput=False)
    wvt = nc.declare_dram_parameter("wvt", [H, 128], BF16, isOutput=False)
    wot = nc.declare_dram_parameter("wot", [H, H], BF16, isOutput=False)
    etab = nc.declare_dram_parameter("etab", [HPC, 128, RCOLS], BF16, isOutput=False)
    bq = nc.declare_dram_parameter("bq", [128], F32, isOutput=False)
    bk = nc.declare_dram_parameter("bk", [128], F32, isOutput=False)
    bv = nc.declare_dram_parameter("bv", [128], F32, isOutput=False)
    ob = nc.declare_dram_parameter("ob", [H], F32, isOutput=False)
    maskb = nc.declare_dram_parameter("maskb", [B, S], F32, isOutput=False)
    idb = nc.declare_dram_parameter("idb", [128, 128], BF16, isOutput=False)
    out = nc.declare_dram_parameter("out", [H, B, 256], F32, isOutput=True)

    units = [(b, hl) for b in range(B) for hl in range(HPC)]  # u = 2*b + hl

    with tile.TileContext(nc) as tc:
        with (
            tc.tile_pool(name="persist", bufs=1) as pp,
            tc.tile_pool(name="dram", bufs=1, space="DRAM") as dram,
            tc.tile_pool(name="psum", bufs=3, space="PSUM") as ps,
        ):
            # ---- persistent constants ----
            e_sb = []
            for hl in range(HPC):
                et = pp.tile([128, RCOLS], BF16, tag=f"etab{hl}", name=f"etab{hl}")
                nc.sync.dma_start(et[:], etab[hl])
                e_sb.append(et)
            maskb_sb = pp.tile([128, B * KC], F32, tag="maskb")
            nc.sync.dma_start(
                maskb_sb[:], maskb[:].rearrange("b (k p) -> p (b k)", p=128)
            )
            bq_sb = pp.tile([128, 1], F32, tag="bq")
            nc.sync.dma_start(bq_sb[:], bq[:].rearrange("(p o) -> p o", o=1))
            bk_sb = pp.tile([128, 1], F32, tag="bk")
            nc.sync.dma_start(bk_sb[:], bk[:].rearrange("(p o) -> p o", o=1))
            bv_sb = pp.tile([128, 1], F32, tag="bv")
            nc.sync.dma_start(bv_sb[:], bv[:].rearrange("(p o) -> p o", o=1))
            ob_sb = pp.tile([128, 8], F32, tag="ob")
            nc.sync.dma_start(ob_sb[:], ob[:].rearrange("(c p) -> p c", p=128))
            id_sb = pp.tile([128, 128], BF16, tag="ident")
            nc.sync.dma_start(id_sb[:], idb[:])
            ones64 = pp.tile([1, 64], F32, tag="ones64")
            nc.vector.memset(ones64[:], 1.0)

            # ---- persistent unit buffers ----
            q_units = []
            k_units = []
            vaug_units = []
            for u, (b, hl) in enumerate(units):
                qu = pp.tile([64, S], BF16, tag=f"qu{u}", name=f"qu{u}")
                ku = pp.tile([64, S], BF16, tag=f"ku{u}", name=f"ku{u}")
                q_units.append(qu)
                k_units.append(ku)
                va = pp.tile([128, 128 * KC], BF16, tag=f"va{u}", name=f"va{u}")
                nc.vector.memset(va[:].bitcast(F32), 0.0)
                vaug_units.append(va)

            with tc.tile_pool(name="proj", bufs=1) as pj:
                # vT per head: [65, TOK]; row 64 = ones (pre-transpose)
                vts = []
                for hl in range(HPC):
                    vt = pj.tile([65, TOK], BF16, tag=f"vt{hl}", name=f"vt{hl}")
                    # 0x3F80 is 1.0 in bf16
                    nc.vector.memset(vt[64:65, :].bitcast(mybir.dt.uint16), 0x3F80)
                    vts.append(vt)
                wqk_sb = []
                wv_sb = []
                for i in range(8):
                    w1 = pj.tile([128, 256], BF16, tag=f"wqk{i}", name=f"wqk{i}")
                    nc.sync.dma_start(w1[:], wqkt[i * 128:(i + 1) * 128, :])
                    wqk_sb.append(w1)
                    w2 = pj.tile([128, 128], BF16, tag=f"wv{i}", name=f"wv{i}")
                    nc.sync.dma_start(w2[:], wvt[i * 128:(i + 1) * 128, :])
                    wv_sb.append(w2)

                # ---- phase 1: qkv projection (transposed) ----
                for s in range(NSPAN):
                    b = s // QSPANS
                    sc = s % QSPANS
                    xts = []
                    for k8 in range(8):
                        xt_sb = pj.tile([128, SPAN], BF16,
                                        tag=f"xts{k8}_{s % 2}",
                                        name=f"xts{k8}_{s}")
                        nc.sync.dma_start(
                            xt_sb[:],
                            xt[k8 * 128:(k8 + 1) * 128, s * SPAN:(s + 1) * SPAN],
                        )
                        xts.append(xt_sb)
                    qp = ps.tile([128, SPAN], F32, tag="mm512")
                    for k8 in range(8):
                        nc.tensor.matmul(qp[:], wqk_sb[k8][:, 0:128], xts[k8][:],
                                         start=(k8 == 0), stop=(k8 == 7))
                    kp = ps.tile([128, SPAN], F32, tag="mm512")
                    for k8 in range(8):
                        nc.tensor.matmul(kp[:], wqk_sb[k8][:, 128:256], xts[k8][:],
                                         start=(k8 == 0), stop=(k8 == 7))
                    vp = ps.tile([128, SPAN], F32, tag="mm512")
                    for k8 in range(8):
                        nc.tensor.matmul(vp[:], wv_sb[k8][:], xts[k8][:],
                                         start=(k8 == 0), stop=(k8 == 7))
                    cols = slice(sc * SPAN, (sc + 1) * SPAN)
                    tcols = slice(s * SPAN, (s + 1) * SPAN)
                    for hl in range(HPC):
                        rows = slice(hl * 64, (hl + 1) * 64)
                        u = 2 * b + hl
                        nc.scalar.activation(q_units[u][:, cols], qp[rows, :],
                                             Ident, bias=bq_sb[rows, :])
                        nc.scalar.activation(k_units[u][:, cols], kp[rows, :],
                                             Ident, bias=bk_sb[rows, :])
                        nc.scalar.activation(vts[hl][0:64, tcols], vp[rows, :],
                                             Ident, bias=bv_sb[rows, :])

                # ---- phase 2: v transposes -> v_aug cols [0,65) of 128 ----
                for u, (b, hl) in enumerate(units):
                    for kc in range(KC):
                        tin = vts[hl][0:65, b * S + kc * 128: b * S + (kc + 1) * 128]
                        pt = ps.tile([128, 65], BF16, tag="small", bufs=2)
                        nc.tensor.transpose(pt[:], tin, id_sb[0:65, 0:65])
                        nc.vector.tensor_copy(
                            vaug_units[u][:, kc * 128: kc * 128 + 65], pt[:])

            # ---- phase 3+: attention + A2A + out-proj ----
            with tc.tile_pool(name="opool", bufs=1) as op:
                wot_sb = []
                for i in range(8):
                    w = op.tile([128, H], BF16, tag=f"wot{i}", name=f"wot{i}")
                    nc.sync.dma_start(w[:], wot[i * 128:(i + 1) * 128, :])
                    wot_sb.append(w)

                a2a_in = [dram.tile([N_CORES, 128, 256], BF16, tag=f"a2ai{b}",
                                    name=f"a2a_in{b}") for b in range(B)]
                a2a_out = [dram.tile([N_CORES, 128, 256], BF16, tag=f"a2ao{b}",
                                     name=f"a2a_out{b}") for b in range(B)]

                with tc.tile_pool(name="attn", bufs=3) as at:
                    AHEAD = 2

                    def norm_chain(pv, b, hl, q0):
                        # deferred normalization: emitted after the NEXT span's
                        # first scores so the PE doesn't stall on the DVE/ACT
                        # recip chain.
                        sums_sb = at.tile([1, SPAN], F32, tag="sums",
                                          name="sums_sb")
                        nc.scalar.copy(sums_sb[:], pv[64:65, :])
                        recip = at.tile([1, SPAN], F32, tag="recip",
                                        name="recip")
                        nc.vector.reciprocal_approx_fast(recip[:], sums_sb[:])
                        rb_ps = ps.tile([64, SPAN], F32, tag="recipb", bufs=1,
                                        name="rb_ps")
                        nc.tensor.matmul(rb_ps[:], ones64[:], recip[:],
                                         start=True, stop=True)
                        recipb = at.tile([64, SPAN], F32, tag="recipb_sb",
                                         name="recipb")
                        nc.scalar.copy(recipb[:], rb_ps[:])
                        normd = at.tile([64, SPAN], BF16, tag="normd",
                                        name="normd")
                        nc.vector.tensor_mul(normd[:], pv[0:64, :], recipb[:])
                        d0 = q0 // 256
                        rows = slice(hl * 64, (hl + 1) * 64)
                        nc.sync.dma_start(a2a_in[b][d0, rows, :],
                                          normd[:, 0:256])
                        nc.sync.dma_start(a2a_in[b][d0 + 1, rows, :],
                                          normd[:, 256:512])

                    def emit_scores(ku, qu, q0, kc):
                        scp = ps.tile([128, SPAN], F32, tag="mm512", name="scp")
                        nc.tensor.matmul(
                            scp[:], ku[:, kc * 128:(kc + 1) * 128],
                            qu[:, q0:q0 + SPAN], start=True, stop=True)
                        return scp

                    pending = None   # (pv, b, hl, q0) of the previous span
                    for u, (b, hl) in enumerate(units):
                        qu, ku, va = q_units[u], k_units[u], vaug_units[u]
                        for sp in range(QSPANS):
                            q0 = sp * SPAN
                            pv = ps.tile([128, SPAN], F32, tag="pv", bufs=2,
                                         name="pv")
                            scps = {}
                            for kc in range(min(AHEAD, KC)):
                                scps[kc] = emit_scores(ku, qu, q0, kc)
                            if pending is not None:
                                norm_chain(*pending)
                                pending = None
                            for kc in range(KC):
                                if kc + AHEAD < KC:
                                    scps[kc + AHEAD] = emit_scores(
                                        ku, qu, q0, kc + AHEAD)
                                scp = scps.pop(kc)
                                expt = at.tile([128, SPAN], BF16, tag="expt")
                                nc.scalar.activation(
                                    expt[:], scp[:], Exp,
                                    bias=maskb_sb[:, b * KC + kc: b * KC + kc + 1])
                                probs = at.tile([128, SPAN], BF16, tag="probs")
                                roff = RM0 - kc * 128 + q0
                                nc.vector.tensor_mul(
                                    probs[:], expt[:],
                                    e_sb[hl][:, roff:roff + SPAN])
                                nc.tensor.matmul(
                                    pv[:], va[:, kc * 128:(kc + 1) * 128],
                                    probs[:],
                                    start=(kc == 0), stop=(kc == KC - 1))
                            pending = (pv, b, hl, q0)
                        if hl == HPC - 1:
                            if pending is not None:
                                norm_chain(*pending)
                                pending = None
                            nc.gpsimd.collective_compute(
                                "AllToAll", mybir.AluOpType.bypass,
                                replica_groups=[list(range(N_CORES))],
                                ins=[a2a_in[b][:]], outs=[a2a_out[b][:]])

                    # ---- out-proj ----
                    for b in range(B):
                        rhs = []
                        for src in range(N_CORES):
                            t = at.tile([128, 256], BF16, tag=f"rhs{src % 4}",
                                        name=f"rhs{b}_{src}")
                            nc.sync.dma_start(t[:], a2a_out[b][src])
                            rhs.append(t)
                        for oc in range(8):
                            yp = ps.tile([128, 256], F32, tag="small", bufs=2)
                            for src in range(N_CORES):
                                nc.tensor.matmul(
                                    yp[:], wot_sb[src][:, oc * 128:(oc + 1) * 128],
                                    rhs[src][:], start=(src == 0),
                                    stop=(src == N_CORES - 1))
                            ysb = at.tile([128, 256], F32, tag="ysb")
                            nc.scalar.activation(ysb[:], yp[:], Ident,
                                                 bias=ob_sb[:, oc:oc + 1])
                            nc.sync.dma_start(out[oc * 128:(oc + 1) * 128, b, :],
                                              ysb[:])
    nc.compile()
    return nc


import importlib  # noqa: E402
mybir = None


def _ensure_concourse():
    global mybir
    if mybir is None:
        _install_ntff_hook()
        mybir = importlib.import_module("concourse.mybir")


def prep_in_maps(x, attention_mask, qkv_w, qkv_b, out_w, out_b):
    x = np.asarray(x, dtype=np.float32)
    attention_mask = np.asarray(attention_mask)
    qkv_w = np.asarray(qkv_w, dtype=np.float32)
    qkv_b = np.asarray(qkv_b, dtype=np.float32)
    out_w = np.asarray(out_w, dtype=np.float32)
    out_b = np.asarray(out_b, dtype=np.float32)

    xt = np.ascontiguousarray(x.reshape(TOK, H).T).astype(BFNP)    # [H, TOK]
    wot = np.ascontiguousarray(out_w.T).astype(BFNP)               # [H, H]
    maskb = np.where(attention_mask, 0.0, MASK_NEG).astype(np.float32)
    idb = np.eye(128, dtype=np.float32).astype(BFNP)

    p = np.arange(128, dtype=np.float64)[:, None]
    m = np.arange(RCOLS, dtype=np.float64)[None, :]
    rel = np.abs(p - m + RM0)

    in_maps = []
    for c in range(N_CORES):
        hA, hB = HPC * c, HPC * c + 1
        ridx = np.r_[hA * 64:(hA + 1) * 64, hB * 64:(hB + 1) * 64]
        wq = qkv_w[ridx, :] * SCALE
        wk = qkv_w[H + ridx, :]
        wv = qkv_w[2 * H + ridx, :]
        wqkt = np.ascontiguousarray(
            np.concatenate([wq.T, wk.T], axis=1)).astype(BFNP)     # [H, 256]
        wvt = np.ascontiguousarray(wv.T).astype(BFNP)              # [H, 128]
        bqv = (qkv_b[ridx] * SCALE).astype(np.float32)
        bkv = qkv_b[H + ridx].astype(np.float32)
        bvv = qkv_b[2 * H + ridx].astype(np.float32)
        etab = np.stack([
            np.exp(-SLOPES[HPC * c + hl] * rel).astype(np.float32)
            for hl in range(HPC)
        ]).astype(BFNP)                                            # [HPC,128,RCOLS]
        in_maps.append({
            "xt": xt, "wqkt": wqkt, "wvt": wvt, "wot": wot, "etab": etab,
            "bq": bqv, "bk": bkv, "bv": bvv, "ob": out_b,
            "maskb": maskb, "idb": idb,
        })
    return in_maps


def assemble_output(results):
    # results[c]["out"] : [H, B, 256] -> y[b, c*256 + t, o]
    arr = np.stack([np.asarray(results[c]["out"], dtype=np.float32)
                    for c in range(N_CORES)])                      # [8, H, B, 256]
    return np.ascontiguousarray(arr.transpose(2, 0, 3, 1).reshape(B, S, H))


LAST_RESULT = None
_NC_CACHE = None


def kernel(x, attention_mask, qkv_w, qkv_b, out_w, out_b):
    global LAST_RESULT, _NC_CACHE
    _ensure_concourse()
    from concourse.bass_utils import run_bass_kernel_spmd

    if _NC_CACHE is None:
        _NC_CACHE = build_nc()
    nc = _NC_CACHE
    in_maps = prep_in_maps(x, attention_mask, qkv_w, qkv_b, out_w, out_b)
    trace = bool(int(os.environ.get("KERNEL_TRACE", "0")))
    res = run_bass_kernel_spmd(nc, in_maps, core_ids=list(range(N_CORES)),
                               trace=trace)
    LAST_RESULT = res
    return assemble_output(res.results)


# revision 18
# speedup vs baseline: 1.2197x; 1.2197x over previous
"""ALiBi attention (B=2, S=2048, H=1024, 16 heads, d=64, f32) on 8 TRN2 cores.

Sharding: heads (2 per core) for qkv-proj + attention; AllToAll per batch
switches to sequence sharding (256 tokens per core) for the output projection.

v2: all matmuls bf16 (FastWeightLoad + 1 cyc/row), and the ALiBi bias is
applied MULTIPLICATIVELY after exp:  probs = exp(scores + mask) * E  where
E_h = exp(-slope_h * |i-j|) is a precomputed per-head Toeplitz table
E[p, m] = exp(-slope|p - m + 1920|), viewed at offset 1920 - 128*kc + q0 for
each (key-chunk, q-span) tile. exp runs on ACT straight out of PSUM (721ns)
and the bf16 SBUF multiply runs on DVE at 4x mode (335ns) — vs the v1
PSUM-reading scalar_tensor_tensor at 812-3900ns.

Other structure per core:
  - qT/kT per (batch, head) [64, 2048] bf16; vT per head [65, 4096] bf16 with
    a ones row (becomes the ones column of v_aug -> PV matmul also emits the
    softmax normalizer in row 64).
  - attention mask folded in as a per-partition additive exp bias (0/-3e4).
  - normalize rows 0..63 by reciprocal_approx_fast(row 64), broadcast across
    partitions with a ones[1,64] fp32 matmul (is_stationary_onezero).
  - AllToAll (bf16 payload, one per batch) redistributes head-dims -> tokens;
    out-proj yT[o, tok] accumulated over the 8 src dim-blocks, + out_b.
"""
import math
import os
import sys
import types
import numpy as np
import ml_dtypes

B = 2
S = 2048
H = 1024
HEADS = 16
D = 64
N_CORES = 8
HPC = HEADS // N_CORES          # heads per core = 2
TOK = B * S                     # 4096
SPAN = 512
NSPAN = TOK // SPAN             # 8 proj spans (batch-major)
QSPANS = S // SPAN              # 4 q spans per batch
KC = S // 128                   # 16 key chunks per batch
RM0 = 1920                      # Toeplitz table center offset
RCOLS = 3968
SCALE = D ** -0.5
MASK_NEG = -30000.0
BFNP = ml_dtypes.bfloat16


def _install_ntff_hook():
    """The agent image's antenv lacks axon_hooks; register the NTFF profiling
    hook ourselves so run_bass_kernel_spmd(trace=True) works under axon."""
    if "antenv.axon_hooks" in sys.modules:
        return
    try:
        sys.path.insert(0, "/root/.axon_site")
        from trn_agent_boot.trn_boot import _ntff_profile_via_ctypes
        hook = _ntff_profile_via_ctypes("/opt/axon/libaxon_pjrt.so")
        mod = types.ModuleType("antenv.axon_hooks")
        mod.get_axon_ntff_profile_hook = lambda: hook
        mod.set_axon_ntff_profile_hook = lambda h: None
        import antenv
        antenv.axon_hooks = mod
        sys.modules["antenv.axon_hooks"] = mod
    except Exception:
        pass


def get_alibi_slopes(num_heads):
    def pow2_slopes(n):
        start = 2 ** (-(2 ** (-(math.log2(n) - 3))))
        return [start * start ** i for i in range(n)]
    if math.log2(num_heads).is_integer():
        return pow2_slopes(num_heads)
    c = 2 ** math.floor(math.log2(num_heads))
    return pow2_slopes(c) + pow2_slopes(2 * c)[: num_heads - c]


SLOPES = get_alibi_slopes(HEADS)


def build_nc():
    import concourse.mybir as mybir
    import concourse.bacc as bacc
    import concourse.tile as tile

    F32 = mybir.dt.float32
    BF16 = mybir.dt.bfloat16
    Exp = mybir.ActivationFunctionType.Exp
    Ident = mybir.ActivationFunctionType.Identity

    nc = bacc.Bacc(None, target_bir_lowering=False)

    xt = nc.declare_dram_parameter("xt", [H, TOK], BF16, isOutput=False)
    wqkt = nc.declare_dram_parameter("wqkt", [H, 256], BF16, isOutput=False)
    wvt = nc.declare_dram_parameter("wvt", [H, 128], BF16, isOutput=False)
    wot = nc.declare_dram_parameter("wot", [H, H], BF16, isOutput=False)
    etab = nc.declare_dram_parameter("etab", [HPC, 128, RCOLS], BF16, isOutput=False)
    bq = nc.declare_dram_parameter("bq", [128], F32, isOutput=False)
    bk = nc.declare_dram_parameter("bk", [128], F32, isOutput=False)
    bv = nc.declare_dram_parameter("bv", [128], F32, isOutput=False)
    ob = nc.declare_dram_parameter("ob", [H], F32, isOutput=False)
    maskb = nc.declare_dram_parameter("maskb", [B, S], F32, isOutput=False)
    idb = nc.declare_dram_parameter("idb", [128, 128], BF16, isOutput=False)
    out = nc.declare_dram_parameter("out", [H, B, 256], F32, isOutput=True)

    units = [(b, hl) for b in range(B) for hl in range(HPC)]  # u = 2*b + hl

    with tile.TileContext(nc) as tc:
        with (
            tc.tile_pool(name="persist", bufs=1) as pp,
            tc.tile_pool(name="dram", bufs=1, space="DRAM") as dram,
            tc.tile_pool(name="psum", bufs=3, space="PSUM") as ps,
        ):
            # ---- persistent constants ----
            e_sb = []
            for hl in range(HPC):
                et = pp.tile([128, RCOLS], BF16, tag=f"etab{hl}", name=f"etab{hl}")
                nc.sync.dma_start(et[:], etab[hl])
                e_sb.append(et)
            maskb_sb = pp.tile([128, B * KC], F32, tag="maskb")
            nc.sync.dma_start(
                maskb_sb[:], maskb[:].rearrange("b (k p) -> p (b k)", p=128)
            )
            bq_sb = pp.tile([128, 1], F32, tag="bq")
            nc.sync.dma_start(bq_sb[:], bq[:].rearrange("(p o) -> p o", o=1))
            bk_sb = pp.tile([128, 1], F32, tag="bk")
            nc.sync.dma_start(bk_sb[:], bk[:].rearrange("(p o) -> p o", o=1))
            bv_sb = pp.tile([128, 1], F32, tag="bv")
            nc.sync.dma_start(bv_sb[:], bv[:].rearrange("(p o) -> p o", o=1))
            ob_sb = pp.tile([128, 8], F32, tag="ob")
            nc.sync.dma_start(ob_sb[:], ob[:].rearrange("(c p) -> p c", p=128))
            id_sb = pp.tile([128, 128], BF16, tag="ident")
            nc.sync.dma_start(id_sb[:], idb[:])
            ones64 = pp.tile([1, 64], BF16, tag="ones64")
            nc.vector.memset(ones64[:].bitcast(mybir.dt.uint16), 0x3F80)

            # ---- persistent unit buffers ----
            q_units = []
            k_units = []
            vaug_units = []
            for u, (b, hl) in enumerate(units):
                qu = pp.tile([64, S], BF16, tag=f"qu{u}", name=f"qu{u}")
                ku = pp.tile([64, S], BF16, tag=f"ku{u}", name=f"ku{u}")
                q_units.append(qu)
                k_units.append(ku)
                va = pp.tile([128, 128 * KC], BF16, tag=f"va{u}", name=f"va{u}")
                nc.vector.memset(va[:].bitcast(F32), 0.0)
                vaug_units.append(va)

            with tc.tile_pool(name="proj", bufs=1) as pj:
                # vT per head: [65, TOK]; row 64 = ones (pre-transpose)
                vts = []
                for hl in range(HPC):
                    vt = pj.tile([65, TOK], BF16, tag=f"vt{hl}", name=f"vt{hl}")
                    # 0x3F80 is 1.0 in bf16
                    nc.vector.memset(vt[64:65, :].bitcast(mybir.dt.uint16), 0x3F80)
                    vts.append(vt)
                wqk_sb = []
                wv_sb = []
                for i in range(8):
                    w1 = pj.tile([128, 256], BF16, tag=f"wqk{i}", name=f"wqk{i}")
                    nc.sync.dma_start(w1[:], wqkt[i * 128:(i + 1) * 128, :])
                    wqk_sb.append(w1)
                    w2 = pj.tile([128, 128], BF16, tag=f"wv{i}", name=f"wv{i}")
                    nc.sync.dma_start(w2[:], wvt[i * 128:(i + 1) * 128, :])
                    wv_sb.append(w2)

                # ---- phase 1: qkv projection (transposed) ----
                for s in range(NSPAN):
                    b = s // QSPANS
                    sc = s % QSPANS
                    xts = []
                    for k8 in range(8):
                        xt_sb = pj.tile([128, SPAN], BF16,
                                        tag=f"xts{k8}_{s % 2}",
                                        name=f"xts{k8}_{s}")
                        nc.sync.dma_start(
                            xt_sb[:],
                            xt[k8 * 128:(k8 + 1) * 128, s * SPAN:(s + 1) * SPAN],
                        )
                        xts.append(xt_sb)
                    qp = ps.tile([128, SPAN], F32, tag="mm512")
                    for k8 in range(8):
                        nc.tensor.matmul(qp[:], wqk_sb[k8][:, 0:128], xts[k8][:],
                                         start=(k8 == 0), stop=(k8 == 7))
                    kp = ps.tile([128, SPAN], F32, tag="mm512")
                    for k8 in range(8):
                        nc.tensor.matmul(kp[:], wqk_sb[k8][:, 128:256], xts[k8][:],
                                         start=(k8 == 0), stop=(k8 == 7))
                    vp = ps.tile([128, SPAN], F32, tag="mm512")
                    for k8 in range(8):
                        nc.tensor.matmul(vp[:], wv_sb[k8][:], xts[k8][:],
                                         start=(k8 == 0), stop=(k8 == 7))
                    cols = slice(sc * SPAN, (sc + 1) * SPAN)
                    tcols = slice(s * SPAN, (s + 1) * SPAN)
                    for hl in range(HPC):
                        rows = slice(hl * 64, (hl + 1) * 64)
                        u = 2 * b + hl
                        nc.scalar.activation(q_units[u][:, cols], qp[rows, :],
                                             Ident, bias=bq_sb[rows, :])
                        nc.scalar.activation(k_units[u][:, cols], kp[rows, :],
                                             Ident, bias=bk_sb[rows, :])
                        nc.scalar.activation(vts[hl][0:64, tcols], vp[rows, :],
                                             Ident, bias=bv_sb[rows, :])

                # ---- phase 2: v transposes -> v_aug cols [0,65) of 128 ----
                for u, (b, hl) in enumerate(units):
                    for kc in range(KC):
                        tin = vts[hl][0:65, b * S + kc * 128: b * S + (kc + 1) * 128]
                        pt = ps.tile([128, 65], BF16, tag="small", bufs=2)
                        nc.tensor.transpose(pt[:], tin, id_sb[0:65, 0:65])
                        nc.vector.tensor_copy(
                            vaug_units[u][:, kc * 128: kc * 128 + 65], pt[:])

            # ---- phase 3+: attention + A2A + out-proj ----
            with tc.tile_pool(name="opool", bufs=1) as op:
                wot_sb = []
                for i in range(8):
                    w = op.tile([128, H], BF16, tag=f"wot{i}", name=f"wot{i}")
                    nc.sync.dma_start(w[:], wot[i * 128:(i + 1) * 128, :])
                    wot_sb.append(w)

                a2a_in = [dram.tile([N_CORES, 128, 256], BF16, tag=f"a2ai{b}",
                                    name=f"a2a_in{b}") for b in range(B)]
                a2a_out = [dram.tile([N_CORES, 128, 256], BF16, tag=f"a2ao{b}",
                                     name=f"a2a_out{b}") for b in range(B)]

                with tc.tile_pool(name="attn", bufs=3) as at:
                    AHEAD = 2

                    def norm_chain(pv, b, hl, q0):
                        # deferred normalization: emitted after the NEXT span's
                        # first scores so the PE doesn't stall on the DVE/ACT
                        # recip chain.
                        sums_sb = at.tile([1, SPAN], F32, tag="sums",
                                          name="sums_sb")
                        nc.scalar.copy(sums_sb[:], pv[64:65, :])
                        recip = at.tile([1, SPAN], F32, tag="recip",
                                        name="recip")
                        nc.vector.reciprocal_approx_fast(recip[:], sums_sb[:])
                        recip_bf = at.tile([1, SPAN], BF16, tag="recip_bf",
                                           name="recip_bf")
                        nc.scalar.copy(recip_bf[:], recip[:])
                        rb_ps = ps.tile([64, SPAN], F32, tag="recipb", bufs=1,
                                        name="rb_ps")
                        nc.tensor.matmul(rb_ps[:], ones64[:], recip_bf[:],
                                         start=True, stop=True)
                        recipb = at.tile([64, SPAN], F32, tag="recipb_sb",
                                         name="recipb")
                        nc.scalar.copy(recipb[:], rb_ps[:])
                        normd = at.tile([64, SPAN], BF16, tag="normd",
                                        name="normd")
                        nc.vector.tensor_mul(normd[:], pv[0:64, :], recipb[:])
                        d0 = q0 // 256
                        rows = slice(hl * 64, (hl + 1) * 64)
                        nc.sync.dma_start(a2a_in[b][d0, rows, :],
                                          normd[:, 0:256])
                        nc.sync.dma_start(a2a_in[b][d0 + 1, rows, :],
                                          normd[:, 256:512])

                    def emit_scores(ku, qu, q0, kc):
                        scp = ps.tile([128, SPAN], F32, tag="mm512", name="scp")
                        nc.tensor.matmul(
                            scp[:], ku[:, kc * 128:(kc + 1) * 128],
                            qu[:, q0:q0 + SPAN], start=True, stop=True)
                        return scp

                    pending = None   # (pv, b, hl, q0) of the previous span
                    for u, (b, hl) in enumerate(units):
                        qu, ku, va = q_units[u], k_units[u], vaug_units[u]
                        for sp in range(QSPANS):
                            q0 = sp * SPAN
                            pv = ps.tile([128, SPAN], F32, tag="pv", bufs=2,
                                         name="pv")
                            scps = {}
                            for kc in range(min(AHEAD, KC)):
                                scps[kc] = emit_scores(ku, qu, q0, kc)
                            if pending is not None:
                                norm_chain(*pending)
                                pending = None
                            for kc in range(KC):
                                if kc + AHEAD < KC:
                                    scps[kc + AHEAD] = emit_scores(
                                        ku, qu, q0, kc + AHEAD)
                                scp = scps.pop(kc)
                                expt = at.tile([128, SPAN], BF16, tag="expt")
                                nc.scalar.activation(
                                    expt[:], scp[:], Exp,
                                    bias=maskb_sb[:, b * KC + kc: b * KC + kc + 1])
                                probs = at.tile([128, SPAN], BF16, tag="probs")
                                roff = RM0 - kc * 128 + q0
                                nc.vector.tensor_mul(
                                    probs[:], expt[:],
                                    e_sb[hl][:, roff:roff + SPAN])
                                nc.tensor.matmul(
                                    pv[:], va[:, kc * 128:(kc + 1) * 128],
                                    probs[:],
                                    start=(kc == 0), stop=(kc == KC - 1))
                            pending = (pv, b, hl, q0)
                        if hl == HPC - 1:
                            if pending is not None:
                                norm_chain(*pending)
                                pending = None
                            nc.gpsimd.collective_compute(
                                "AllToAll", mybir.AluOpType.bypass,
                                replica_groups=[list(range(N_CORES))],
                                ins=[a2a_in[b][:]], outs=[a2a_out[b][:]])

                    # ---- out-proj ----
                    for b in range(B):
                        rhs = []
                        for src in range(N_CORES):
                            t = at.tile([128, 256], BF16, tag=f"rhs{src % 4}",
                                        name=f"rhs{b}_{src}")
                            nc.sync.dma_start(t[:], a2a_out[b][src])
                            rhs.append(t)
                        for oc in range(8):
                            yp = ps.tile([128, 256], F32, tag="small", bufs=2)
                            for src in range(N_CORES):
                                nc.tensor.matmul(
                                    yp[:], wot_sb[src][:, oc * 128:(oc + 1) * 128],
                                    rhs[src][:], start=(src == 0),
                                    stop=(src == N_CORES - 1))
                            ysb = at.tile([128, 256], F32, tag="ysb")
                            nc.scalar.activation(ysb[:], yp[:], Ident,
                                                 bias=ob_sb[:, oc:oc + 1])
                            nc.sync.dma_start(out[oc * 128:(oc + 1) * 128, b, :],
                                              ysb[:])
    nc.compile()
    return nc


import importlib  # noqa: E402
mybir = None


def _ensure_concourse():
    global mybir
    if mybir is None:
        _install_ntff_hook()
        mybir = importlib.import_module("concourse.mybir")


def prep_in_maps(x, attention_mask, qkv_w, qkv_b, out_w, out_b):
    x = np.asarray(x, dtype=np.float32)
    attention_mask = np.asarray(attention_mask)
    qkv_w = np.asarray(qkv_w, dtype=np.float32)
    qkv_b = np.asarray(qkv_b, dtype=np.float32)
    out_w = np.asarray(out_w, dtype=np.float32)
    out_b = np.asarray(out_b, dtype=np.float32)

    xt = np.ascontiguousarray(x.reshape(TOK, H).T).astype(BFNP)    # [H, TOK]
    wot = np.ascontiguousarray(out_w.T).astype(BFNP)               # [H, H]
    maskb = np.where(attention_mask, 0.0, MASK_NEG).astype(np.float32)
    idb = np.eye(128, dtype=np.float32).astype(BFNP)

    p = np.arange(128, dtype=np.float64)[:, None]
    m = np.arange(RCOLS, dtype=np.float64)[None, :]
    rel = np.abs(p - m + RM0)

    in_maps = []
    for c in range(N_CORES):
        hA, hB = HPC * c, HPC * c + 1
        ridx = np.r_[hA * 64:(hA + 1) * 64, hB * 64:(hB + 1) * 64]
        wq = qkv_w[ridx, :] * SCALE
        wk = qkv_w[H + ridx, :]
        wv = qkv_w[2 * H + ridx, :]
        wqkt = np.ascontiguousarray(
            np.concatenate([wq.T, wk.T], axis=1)).astype(BFNP)     # [H, 256]
        wvt = np.ascontiguousarray(wv.T).astype(BFNP)              # [H, 128]
        bqv = (qkv_b[ridx] * SCALE).astype(np.float32)
        bkv = qkv_b[H + ridx].astype(np.float32)
        bvv = qkv_b[2 * H + ridx].astype(np.float32)
        etab = np.stack([
            np.exp(-SLOPES[HPC * c + hl] * rel).astype(np.float32)
            for hl in range(HPC)
        ]).astype(BFNP)                                            # [HPC,128,RCOLS]
        in_maps.append({
            "xt": xt, "wqkt": wqkt, "wvt": wvt, "wot": wot, "etab": etab,
            "bq": bqv, "bk": bkv, "bv": bvv, "ob": out_b,
            "maskb": maskb, "idb": idb,
        })
    return in_maps


def assemble_output(results):
    # results[c]["out"] : [H, B, 256] -> y[b, c*256 + t, o]
    arr = np.stack([np.asarray(results[c]["out"], dtype=np.float32)
                    for c in range(N_CORES)])                      # [8, H, B, 256]
    return np.ascontiguousarray(arr.transpose(2, 0, 3, 1).reshape(B, S, H))


LAST_RESULT = None
_NC_CACHE = None


def kernel(x, attention_mask, qkv_w, qkv_b, out_w, out_b):
    global LAST_RESULT, _NC_CACHE
    _ensure_concourse()
    from concourse.bass_utils import run_bass_kernel_spmd

    if _NC_CACHE is None:
        _NC_CACHE = build_nc()
    nc = _NC_CACHE
    in_maps = prep_in_maps(x, attention_mask, qkv_w, qkv_b, out_w, out_b)
    trace = bool(int(os.environ.get("KERNEL_TRACE", "0")))
    res = run_bass_kernel_spmd(nc, in_maps, core_ids=list(range(N_CORES)),
                               trace=trace)
    LAST_RESULT = res
    return assemble_output(res.results)
